# revision 15
# baseline (speedup 1.0000x reference)
import sys

sys.path.insert(0, "/opt/trn_rl_repo")
import numpy as np
import ml_dtypes
import concourse.bass as bass
import concourse.bacc as bacc
import concourse.mybir as mybir
import concourse.tile as tile
from concourse import masks
import concourse.bass_utils as bass_utils

bass_utils.upload_artifacts = lambda tmpdir: "local://" + tmpdir
from concourse.bass_utils import run_bass_kernel_spmd

N_CORES = 8
B, H, W, C, R = 32, 56, 56, 256, 16
BS = B // N_CORES          # 4 samples per core
NP = H * W                 # 3136 pixels per sample
PT = 112                   # partitions per tile (2 image rows)
NT = NP // PT              # 28 tiles per sample
TPS = 7                    # tiles per DMA slab
NSLAB = NT // TPS          # 4 slabs per sample
ROWS = BS * NP             # 12544 rows per core
F32 = mybir.dt.float32
BF16 = mybir.dt.bfloat16
AL = mybir.AluOpType
AF = mybir.ActivationFunctionType
AX = mybir.AxisListType

_COMPILED = None


def _build():
    nc = bacc.Bacc(None, target_bir_lowering=False, num_devices=N_CORES)
    x_d = nc.declare_dram_parameter("x", [ROWS, C], BF16, isOutput=False)
    w1_d = nc.declare_dram_parameter("w1", [C, R], F32, isOutput=False)
    b1_d = nc.declare_dram_parameter("b1", [1, R], F32, isOutput=False)
    w2_d = nc.declare_dram_parameter("w2", [R, C], F32, isOutput=False)
    b2_d = nc.declare_dram_parameter("b2", [1, C], F32, isOutput=False)
    wv_d = nc.declare_dram_parameter("wv", [14 * 62, 56], F32, isOutput=False)
    bc_d = nc.declare_dram_parameter("bconv", [1, 1], F32, isOutput=False)
    out_d = nc.declare_dram_parameter("out", [ROWS, C], BF16, isOutput=True)
    # DRAM bounce buffers for plane-layout rearrangement (ping-pong x2)
    sd_dram = nc.dram_tensor("sd_dram", [BS * 2, NP], F32)
    sc_dram = nc.dram_tensor("sc_dram", [BS, NP], F32)

    with tile.TileContext(nc) as tc:
        with tc.tile_pool(name="const", bufs=1) as cp, \
             tc.tile_pool(name="xbuf", bufs=1) as xp, \
             tc.tile_pool(name="work", bufs=3) as wp, \
             tc.tile_pool(name="sp", bufs=3) as spp, \
             tc.tile_pool(name="psPool", bufs=2, space="PSUM") as psP, \
             tc.tile_pool(name="psA", bufs=2, space="PSUM") as psA, \
             tc.tile_pool(name="psB", bufs=2, space="PSUM") as psB, \
             tc.tile_pool(name="psC", bufs=2, space="PSUM") as psC:

            # ---------- constants ----------
            ident_f = cp.tile([128, 128], F32)
            masks.make_identity(nc, ident_f[:])
            ident_b = cp.tile([128, 128], BF16)
            masks.make_identity(nc, ident_b[:])
            ones_t = cp.tile([PT, 1], BF16)
            nc.gpsimd.memset(ones_t[:], 1.0)
            ones2f = cp.tile([2, PT], F32)
            nc.gpsimd.memset(ones2f[:], 1.0)

            w1t = cp.tile([128, 2 * R], F32)       # [K-chunk, 2*16]
            nc.sync.dma_start(w1t[:, 0:R], w1_d[0:128, :])
            nc.sync.dma_start(w1t[:, R:2 * R], w1_d[128:256, :])
            w2t = cp.tile([R, C], F32)
            nc.sync.dma_start(w2t[:], w2_d[:])
            # 14 band matrices [62,56], one per (channel, dx)
            wv_sb = cp.tile([62, 14, 56], F32)
            nc.sync.dma_start(
                wv_sb[:],
                bass.AP(wv_d, 0, [[56, 62], [62 * 56, 14], [1, 56]]))

            b1r = cp.tile([1, R], F32)
            nc.sync.dma_start(b1r[:], b1_d[:])
            b1b = cp.tile([2, R], F32)
            nc.gpsimd.partition_broadcast(b1b[:], b1r[:], channels=2)
            b2r = cp.tile([1, C], F32)
            nc.sync.dma_start(b2r[:], b2_d[:])
            b2b = cp.tile([2, C], F32)
            nc.gpsimd.partition_broadcast(b2b[:], b2r[:], channels=2)
            bcr = cp.tile([1, 1], F32)
            nc.sync.dma_start(bcr[:], bc_d[:])
            bcb = cp.tile([56, 1], F32)
            nc.gpsimd.partition_broadcast(bcb[:], bcr[:], channels=56)

            # zero-padded conv input planes (borders stay zero), ping-pong x2
            pads = []
            for i in range(BS):
                pm = cp.tile([62, 56], F32, name=f"padm{i}")
                px = cp.tile([62, 56], F32, name=f"padx{i}")
                nc.vector.memset(pm[:], 0.0)
                nc.vector.memset(px[:], 0.0)
                pads.append((pm, px))

            # resident x (overwritten in place by xg then by out)
            xbuf = xp.tile([PT, BS * NT, C], BF16)

            # ---------- load all of x ----------
            for s in range(BS):
                for j in range(NSLAB):
                    base = (s * NP + j * TPS * PT) * C
                    nc.sync.dma_start(
                        xbuf[:, s * NT + j * TPS:s * NT + (j + 1) * TPS, :],
                        bass.AP(x_d, base, [[C, PT], [PT * C, TPS], [1, C]]))

            for s in range(BS):
                # ---------- channel pooling ----------
                pool_ps = psP.tile([1, C], F32, tag="pool")
                for t in range(NT):
                    v = xbuf[:, s * NT + t, :]
                    nc.tensor.matmul(
                        pool_ps[:], ones_t[:], v,
                        start=(t == 0), stop=(t == NT - 1),
                        skip_group_check=True)
                scr = wp.tile([PT, 14, C], BF16, tag="scr")
                nc.vector.tensor_tensor(
                    out=scr[:], in0=xbuf[:, s * NT:s * NT + 14, :],
                    in1=xbuf[:, s * NT + 14:s * NT + 28, :], op=AL.max)
                s7 = wp.tile([PT, 7, C], BF16, tag="s7")
                nc.vector.tensor_tensor(
                    out=s7[:], in0=scr[:, 0:7, :], in1=scr[:, 7:14, :],
                    op=AL.max)
                maxacc = wp.tile([PT, C], BF16, tag="maxacc")
                nc.vector.tensor_tensor(out=maxacc[:], in0=s7[:, 0, :],
                                        in1=s7[:, 1, :], op=AL.max)
                for k in range(2, 7):
                    nc.vector.tensor_tensor(out=maxacc[:], in0=maxacc[:],
                                            in1=s7[:, k, :], op=AL.max)

                # ---------- channel MLP ----------
                poolsb = wp.tile([1, C], F32, tag="poolsb")
                nc.scalar.activation(poolsb[:], pool_ps[:],
                                     AF.Copy, scale=1.0 / NP)
                rhs_s = wp.tile([128, 2, 2], F32, tag="rhs")
                for c in range(2):
                    tp = psB.tile([128, 1], F32, tag="psb")
                    nc.tensor.transpose(tp[:], poolsb[:, c * 128:(c + 1) * 128],
                                        ident_f[0:1, 0:1])
                    nc.vector.tensor_copy(rhs_s[:, c, 0:1], tp[:])
                    mt = psA.tile([128, PT], BF16, tag="psa")
                    nc.tensor.transpose(mt[:], maxacc[:, c * 128:(c + 1) * 128],
                                        ident_b[0:PT, 0:PT])
                    nc.vector.reduce_max(rhs_s[:, c, 1:2], mt[:], axis=AX.X)
                h_ps = psB.tile([2, R], F32, tag="psb")
                nc.tensor.matmul(h_ps[:], rhs_s[:, 0, :], w1t[:, 0:R],
                                 start=True, stop=False)
                nc.tensor.matmul(h_ps[:], rhs_s[:, 1, :], w1t[:, R:2 * R],
                                 start=False, stop=True)
                hb = wp.tile([2, R], F32, tag="hb")
                nc.vector.tensor_tensor(out=hb[:], in0=h_ps[:], in1=b1b[:],
                                        op=AL.add)
                hr = wp.tile([2, R], F32, tag="hr")
                nc.scalar.activation(hr[:], hb[:], AF.Relu)
                hT_ps = psB.tile([R, 2], F32, tag="psb")
                nc.tensor.transpose(hT_ps[:], hr[:], ident_f[0:2, 0:2])
                hT = wp.tile([R, 2], F32, tag="hT")
                nc.vector.tensor_copy(hT[:], hT_ps[:])
                co_ps = psB.tile([2, C], F32, tag="psb")
                nc.tensor.matmul(co_ps[:], hT[:], w2t[:], start=True, stop=True)
                co_sb = wp.tile([2, C], F32, tag="co")
                nc.vector.tensor_tensor(out=co_sb[:], in0=co_ps[:], in1=b2b[:],
                                        op=AL.add)
                sig = wp.tile([2, C], F32, tag="sig")
                nc.scalar.activation(sig[:], co_sb[:], AF.Sigmoid)
                cb_ps = psB.tile([PT, C], F32, tag="psb")
                nc.tensor.matmul(cb_ps[:], ones2f[:], sig[:],
                                 start=True, stop=True)
                cbb = wp.tile([PT, C], BF16, tag="cbb")
                nc.vector.tensor_copy(cbb[:], cb_ps[:])

                # ---------- xg (in place) + spatial stats ----------
                spx = spp.tile([PT, NT], F32, tag="spx")
                spm = spp.tile([PT, NT], F32, tag="spm")
                for t in range(NT):
                    v = xbuf[:, s * NT + t, :]
                    nc.vector.tensor_tensor(out=v, in0=v, in1=cbb[:],
                                            op=AL.mult)
                for j in range(NSLAB):
                    slab = xbuf[:, s * NT + j * TPS:s * NT + (j + 1) * TPS, :]
                    nc.vector.reduce_max(spx[:, j * TPS:(j + 1) * TPS],
                                         slab, axis=AX.X)
                    nc.vector.reduce_sum(spm[:, j * TPS:(j + 1) * TPS],
                                         slab, axis=AX.X)

                # ---------- 7x7x2 conv via banded matmuls ----------
                padm, padx = pads[s]
                for ci, (plane, padt) in enumerate(((spm, padm), (spx, padx))):
                    tps = psA.tile([NT, PT], F32, tag="psa")
                    nc.tensor.transpose(tps[:], plane[:], ident_f[0:PT, 0:PT])
                    smT = wp.tile([NT, PT], F32, tag="smT")
                    nc.vector.tensor_copy(smT[:], tps[:])
                    row = s * 2 + ci
                    nc.sync.dma_start(
                        bass.AP(sd_dram, row * NP, [[112, 28], [1, 112]]),
                        smT[:])
                    nc.sync.dma_start(
                        padt[3:59, :],
                        bass.AP(sd_dram, row * NP, [[56, 56], [1, 56]]))
                conv_ps = psC.tile([56, 56], F32, tag="conv")
                dx_order = [3, 0, 1, 2, 4, 5, 6]
                nmm = 0
                for c, padt in ((0, padm), (1, padx)):
                    for dx in (dx_order if c == 0 else range(7)):
                        d = dx - 3
                        a = max(0, -d)
                        b = 56 - max(0, d)
                        nc.tensor.matmul(
                            conv_ps[0:56, a:b], wv_sb[:, c * 7 + dx, :],
                            padt[:, a + d:b + d],
                            start=(nmm == 0), stop=(nmm == 13),
                            skip_group_check=True)
                        nmm += 1
                spsc_yx = wp.tile([56, 56], F32, tag="spscyx")
                nc.scalar.activation(spsc_yx[:], conv_ps[:], AF.Sigmoid,
                                     bias=bcb[:])
                spscT = wp.tile([NT, PT], F32, tag="spscT")
                nc.sync.dma_start(
                    bass.AP(sc_dram, s * NP, [[1, NP]]), spsc_yx[:])
                nc.sync.dma_start(
                    spscT[:],
                    bass.AP(sc_dram, s * NP, [[112, 28], [1, 112]]))
                tps2 = psA.tile([PT, NT], F32, tag="psa")
                nc.tensor.transpose(tps2[:], spscT[:], ident_f[0:NT, 0:NT])
                spsc = spp.tile([PT, NT], F32, tag="spsc")
                nc.vector.tensor_copy(spsc[:], tps2[:])

                # ---------- out = xg * spatial (in place) + store ----------
                for t in range(NT):
                    v = xbuf[:, s * NT + t, :]
                    nc.scalar.activation(v, v, AF.Copy,
                                         scale=spsc[:, t:t + 1])
                for j in range(NSLAB):
                    base = (s * NP + j * TPS * PT) * C
                    nc.sync.dma_start(
                        bass.AP(out_d, base, [[C, PT], [PT * C, TPS], [1, C]]),
                        xbuf[:, s * NT + j * TPS:s * NT + (j + 1) * TPS, :])

    nc.compile()
    return nc


def _get_compiled():
    global _COMPILED
    if _COMPILED is None:
        _COMPILED = _build()
    return _COMPILED


def _make_wv(wconv):
    # wv[(c*7+dx)*62 + y', x] = wconv[y'-y, dx, c, 0]  (banded, SAME pad in y)
    w = np.asarray(wconv, dtype=np.float32)[:, :, :, 0]    # [dy, dx, c]
    w = w.copy()
    w[:, :, 0] /= C       # fold channel-mean 1/256 into the mean-plane taps
    wv = np.zeros((14, 62, 56), dtype=np.float32)
    idx = np.arange(56)
    for c in range(2):
        for dx in range(7):
            for dy in range(7):
                wv[c * 7 + dx, idx + dy, idx] = w[dy, dx, c]
    return wv.reshape(14 * 62, 56)


def kernel(x, w1, b1, w2, b2, wconv, bconv):
    x = np.asarray(x, dtype=np.float32).reshape(N_CORES, ROWS, C)
    xbf = x.astype(ml_dtypes.bfloat16)
    wv = _make_wv(wconv)

    nc = _get_compiled()
    in_maps = [{
        "x": np.ascontiguousarray(xbf[i]),
        "w1": np.asarray(w1, np.float32),
        "b1": np.asarray(b1, np.float32).reshape(1, R),
        "w2": np.asarray(w2, np.float32),
        "b2": np.asarray(b2, np.float32).reshape(1, C),
        "wv": wv,
        "bconv": np.asarray(bconv, np.float32).reshape(1, 1),
    } for i in range(N_CORES)]
    res = run_bass_kernel_spmd(nc, in_maps, list(range(N_CORES)))
    out = np.stack([np.asarray(res.results[i]["out"]) for i in range(N_CORES)],
                   axis=0)
    return out.astype(np.float32).reshape(B, H, W, C)


# revision 16
# speedup vs baseline: 1.0009x; 1.0009x over previous
import sys

sys.path.insert(0, "/opt/trn_rl_repo")
import numpy as np
import ml_dtypes
import concourse.bass as bass
import concourse.bacc as bacc
import concourse.mybir as mybir
import concourse.tile as tile
from concourse import masks
import concourse.bass_utils as bass_utils

bass_utils.upload_artifacts = lambda tmpdir: "local://" + tmpdir
from concourse.bass_utils import run_bass_kernel_spmd

N_CORES = 8
B, H, W, C, R = 32, 56, 56, 256, 16
BS = B // N_CORES          # 4 samples per core
NP = H * W                 # 3136 pixels per sample
PT = 112                   # partitions per tile (2 image rows)
NT = NP // PT              # 28 tiles per sample
TPS = 7                    # tiles per DMA slab
NSLAB = NT // TPS          # 4 slabs per sample
ROWS = BS * NP             # 12544 rows per core
F32 = mybir.dt.float32
BF16 = mybir.dt.bfloat16
AL = mybir.AluOpType
AF = mybir.ActivationFunctionType
AX = mybir.AxisListType

_COMPILED = None


def _build():
    nc = bacc.Bacc(None, target_bir_lowering=False, num_devices=N_CORES)
    x_d = nc.declare_dram_parameter("x", [ROWS, C], BF16, isOutput=False)
    w1_d = nc.declare_dram_parameter("w1", [C, R], F32, isOutput=False)
    b1_d = nc.declare_dram_parameter("b1", [1, R], F32, isOutput=False)
    w2_d = nc.declare_dram_parameter("w2", [R, C], F32, isOutput=False)
    b2_d = nc.declare_dram_parameter("b2", [1, C], F32, isOutput=False)
    wv_d = nc.declare_dram_parameter("wv", [14 * 62, 56], F32, isOutput=False)
    bc_d = nc.declare_dram_parameter("bconv", [1, 1], F32, isOutput=False)
    out_d = nc.declare_dram_parameter("out", [ROWS, C], BF16, isOutput=True)
    # DRAM bounce buffers for plane-layout rearrangement (ping-pong x2)
    sd_dram = nc.dram_tensor("sd_dram", [BS * 2, NP], F32)
    sc_dram = nc.dram_tensor("sc_dram", [BS, NP], F32)

    with tile.TileContext(nc) as tc:
        with tc.tile_pool(name="const", bufs=1) as cp, \
             tc.tile_pool(name="xbuf", bufs=1) as xp, \
             tc.tile_pool(name="work", bufs=3) as wp, \
             tc.tile_pool(name="sp", bufs=3) as spp, \
             tc.tile_pool(name="psPool", bufs=2, space="PSUM") as psP, \
             tc.tile_pool(name="psA", bufs=2, space="PSUM") as psA, \
             tc.tile_pool(name="psB", bufs=2, space="PSUM") as psB, \
             tc.tile_pool(name="psC", bufs=2, space="PSUM") as psC:

            # ---------- constants ----------
            ident_f = cp.tile([128, 128], F32)
            masks.make_identity(nc, ident_f[:])
            ident_b = cp.tile([128, 128], BF16)
            masks.make_identity(nc, ident_b[:])
            ones_t = cp.tile([PT, 1], BF16)
            nc.gpsimd.memset(ones_t[:], 1.0)
            ones2f = cp.tile([2, PT], F32)
            nc.gpsimd.memset(ones2f[:], 1.0)

            w1t = cp.tile([128, 2 * R], F32)       # [K-chunk, 2*16]
            nc.sync.dma_start(w1t[:, 0:R], w1_d[0:128, :])
            nc.sync.dma_start(w1t[:, R:2 * R], w1_d[128:256, :])
            w2t = cp.tile([R, C], F32)
            nc.sync.dma_start(w2t[:], w2_d[:])
            # 14 band matrices [62,56], one per (channel, dx)
            wv_sb = cp.tile([62, 14, 56], F32)
            nc.sync.dma_start(
                wv_sb[:],
                bass.AP(wv_d, 0, [[56, 62], [62 * 56, 14], [1, 56]]))

            b1r = cp.tile([1, R], F32)
            nc.sync.dma_start(b1r[:], b1_d[:])
            b1b = cp.tile([2, R], F32)
            nc.gpsimd.partition_broadcast(b1b[:], b1r[:], channels=2)
            b2r = cp.tile([1, C], F32)
            nc.sync.dma_start(b2r[:], b2_d[:])
            b2b = cp.tile([2, C], F32)
            nc.gpsimd.partition_broadcast(b2b[:], b2r[:], channels=2)
            bcr = cp.tile([1, 1], F32)
            nc.sync.dma_start(bcr[:], bc_d[:])
            bcb = cp.tile([56, 1], F32)
            nc.gpsimd.partition_broadcast(bcb[:], bcr[:], channels=56)

            # zero-padded conv input planes (borders stay zero), ping-pong x2
            pads = []
            for i in range(BS):
                pm = cp.tile([62, 56], F32, name=f"padm{i}")
                px = cp.tile([62, 56], F32, name=f"padx{i}")
                nc.vector.memset(pm[:], 0.0)
                nc.vector.memset(px[:], 0.0)
                pads.append((pm, px))

            # resident x, one tile per sample (overwritten by xg then out)
            xbufs = [xp.tile([PT, NT, C], BF16, name=f"xb{i}")
                     for i in range(BS)]

            # ---------- load all of x ----------
            for s in range(BS):
                for j in range(NSLAB):
                    base = (s * NP + j * TPS * PT) * C
                    nc.sync.dma_start(
                        xbufs[s][:, j * TPS:(j + 1) * TPS, :],
                        bass.AP(x_d, base, [[C, PT], [PT * C, TPS], [1, C]]))

            for s in range(BS):
                xb = xbufs[s]
                # ---------- channel pooling ----------
                pool_ps = psP.tile([1, C], F32, tag="pool")
                for t in range(NT):
                    v = xb[:, t, :]
                    nc.tensor.matmul(
                        pool_ps[:], ones_t[:], v,
                        start=(t == 0), stop=(t == NT - 1),
                        skip_group_check=True)
                scr = wp.tile([PT, 14, C], BF16, tag="scr")
                nc.vector.tensor_tensor(
                    out=scr[:], in0=xb[:, 0:14, :],
                    in1=xb[:, 14:28, :], op=AL.max)
                s7 = wp.tile([PT, 7, C], BF16, tag="s7")
                nc.vector.tensor_tensor(
                    out=s7[:], in0=scr[:, 0:7, :], in1=scr[:, 7:14, :],
                    op=AL.max)
                maxacc = wp.tile([PT, C], BF16, tag="maxacc")
                nc.vector.tensor_tensor(out=maxacc[:], in0=s7[:, 0, :],
                                        in1=s7[:, 1, :], op=AL.max)
                for k in range(2, 7):
                    nc.vector.tensor_tensor(out=maxacc[:], in0=maxacc[:],
                                            in1=s7[:, k, :], op=AL.max)

                # ---------- channel MLP ----------
                poolsb = wp.tile([1, C], F32, tag="poolsb")
                nc.scalar.activation(poolsb[:], pool_ps[:],
                                     AF.Copy, scale=1.0 / NP)
                rhs_s = wp.tile([128, 2, 2], F32, tag="rhs")
                for c in range(2):
                    tp = psB.tile([128, 1], F32, tag="psb")
                    nc.tensor.transpose(tp[:], poolsb[:, c * 128:(c + 1) * 128],
                                        ident_f[0:1, 0:1])
                    nc.vector.tensor_copy(rhs_s[:, c, 0:1], tp[:])
                    mt = psA.tile([128, PT], BF16, tag="psa")
                    nc.tensor.transpose(mt[:], maxacc[:, c * 128:(c + 1) * 128],
                                        ident_b[0:PT, 0:PT])
                    nc.vector.reduce_max(rhs_s[:, c, 1:2], mt[:], axis=AX.X)
                h_ps = psB.tile([2, R], F32, tag="psb")
                nc.tensor.matmul(h_ps[:], rhs_s[:, 0, :], w1t[:, 0:R],
                                 start=True, stop=False)
                nc.tensor.matmul(h_ps[:], rhs_s[:, 1, :], w1t[:, R:2 * R],
                                 start=False, stop=True)
                hb = wp.tile([2, R], F32, tag="hb")
                nc.vector.tensor_tensor(out=hb[:], in0=h_ps[:], in1=b1b[:],
                                        op=AL.add)
                hr = wp.tile([2, R], F32, tag="hr")
                nc.scalar.activation(hr[:], hb[:], AF.Relu)
                hT_ps = psB.tile([R, 2], F32, tag="psb")
                nc.tensor.transpose(hT_ps[:], hr[:], ident_f[0:2, 0:2])
                hT = wp.tile([R, 2], F32, tag="hT")
                nc.vector.tensor_copy(hT[:], hT_ps[:])
                co_ps = psB.tile([2, C], F32, tag="psb")
                nc.tensor.matmul(co_ps[:], hT[:], w2t[:], start=True, stop=True)
                co_sb = wp.tile([2, C], F32, tag="co")
                nc.vector.tensor_tensor(out=co_sb[:], in0=co_ps[:], in1=b2b[:],
                                        op=AL.add)
                sig = wp.tile([2, C], F32, tag="sig")
                nc.scalar.activation(sig[:], co_sb[:], AF.Sigmoid)
                cb_ps = psB.tile([PT, C], F32, tag="psb")
                nc.tensor.matmul(cb_ps[:], ones2f[:], sig[:],
                                 start=True, stop=True)
                cbb = wp.tile([PT, C], BF16, tag="cbb")
                nc.vector.tensor_copy(cbb[:], cb_ps[:])

                # ---------- xg (in place) + spatial stats ----------
                spx = spp.tile([PT, NT], F32, tag="spx")
                spm = spp.tile([PT, NT], F32, tag="spm")
                for t in range(NT):
                    v = xb[:, t, :]
                    nc.vector.tensor_tensor(out=v, in0=v, in1=cbb[:],
                                            op=AL.mult)
                for j in range(NSLAB):
                    slab = xb[:, j * TPS:(j + 1) * TPS, :]
                    nc.vector.reduce_max(spx[:, j * TPS:(j + 1) * TPS],
                                         slab, axis=AX.X)
                    nc.vector.reduce_sum(spm[:, j * TPS:(j + 1) * TPS],
                                         slab, axis=AX.X)

                # ---------- 7x7x2 conv via banded matmuls ----------
                padm, padx = pads[s]
                for ci, (plane, padt) in enumerate(((spm, padm), (spx, padx))):
                    tps = psA.tile([NT, PT], F32, tag="psa")
                    nc.tensor.transpose(tps[:], plane[:], ident_f[0:PT, 0:PT])
                    smT = wp.tile([NT, PT], F32, tag="smT")
                    nc.vector.tensor_copy(smT[:], tps[:])
                    row = s * 2 + ci
                    nc.sync.dma_start(
                        bass.AP(sd_dram, row * NP, [[112, 28], [1, 112]]),
                        smT[:])
                    nc.sync.dma_start(
                        padt[3:59, :],
                        bass.AP(sd_dram, row * NP, [[56, 56], [1, 56]]))
                conv_ps = psC.tile([56, 56], F32, tag="conv")
                dx_order = [3, 0, 1, 2, 4, 5, 6]
                nmm = 0
                for c, padt in ((0, padm), (1, padx)):
                    for dx in (dx_order if c == 0 else range(7)):
                        d = dx - 3
                        a = max(0, -d)
                        b = 56 - max(0, d)
                        nc.tensor.matmul(
                            conv_ps[0:56, a:b], wv_sb[:, c * 7 + dx, :],
                            padt[:, a + d:b + d],
                            start=(nmm == 0), stop=(nmm == 13),
                            skip_group_check=True)
                        nmm += 1
                spsc_yx = wp.tile([56, 56], F32, tag="spscyx")
                nc.scalar.activation(spsc_yx[:], conv_ps[:], AF.Sigmoid,
                                     bias=bcb[:])
                spscT = wp.tile([NT, PT], F32, tag="spscT")
                nc.sync.dma_start(
                    bass.AP(sc_dram, s * NP, [[1, NP]]), spsc_yx[:])
                nc.sync.dma_start(
                    spscT[:],
                    bass.AP(sc_dram, s * NP, [[112, 28], [1, 112]]))
                tps2 = psA.tile([PT, NT], F32, tag="psa")
                nc.tensor.transpose(tps2[:], spscT[:], ident_f[0:NT, 0:NT])
                spsc = spp.tile([PT, NT], F32, tag="spsc")
                nc.vector.tensor_copy(spsc[:], tps2[:])

                # ---------- out = xg * spatial (in place) + store ----------
                for t in range(NT):
                    v = xb[:, t, :]
                    nc.scalar.activation(v, v, AF.Copy,
                                         scale=spsc[:, t:t + 1])
                for j in range(NSLAB):
                    base = (s * NP + j * TPS * PT) * C
                    nc.sync.dma_start(
                        bass.AP(out_d, base, [[C, PT], [PT * C, TPS], [1, C]]),
                        xb[:, j * TPS:(j + 1) * TPS, :])

    nc.compile()
    return nc


def _get_compiled():
    global _COMPILED
    if _COMPILED is None:
        _COMPILED = _build()
    return _COMPILED


def _make_wv(wconv):
    # wv[(c*7+dx)*62 + y', x] = wconv[y'-y, dx, c, 0]  (banded, SAME pad in y)
    w = np.asarray(wconv, dtype=np.float32)[:, :, :, 0]    # [dy, dx, c]
    w = w.copy()
    w[:, :, 0] /= C       # fold channel-mean 1/256 into the mean-plane taps
    wv = np.zeros((14, 62, 56), dtype=np.float32)
    idx = np.arange(56)
    for c in range(2):
        for dx in range(7):
            for dy in range(7):
                wv[c * 7 + dx, idx + dy, idx] = w[dy, dx, c]
    return wv.reshape(14 * 62, 56)


def kernel(x, w1, b1, w2, b2, wconv, bconv):
    x = np.asarray(x, dtype=np.float32).reshape(N_CORES, ROWS, C)
    xbf = x.astype(ml_dtypes.bfloat16)
    wv = _make_wv(wconv)

    nc = _get_compiled()
    in_maps = [{
        "x": np.ascontiguousarray(xbf[i]),
        "w1": np.asarray(w1, np.float32),
        "b1": np.asarray(b1, np.float32).reshape(1, R),
        "w2": np.asarray(w2, np.float32),
        "b2": np.asarray(b2, np.float32).reshape(1, C),
        "wv": wv,
        "bconv": np.asarray(bconv, np.float32).reshape(1, 1),
    } for i in range(N_CORES)]
    res = run_bass_kernel_spmd(nc, in_maps, list(range(N_CORES)))
    out = np.stack([np.asarray(res.results[i]["out"]) for i in range(N_CORES)],
                   axis=0)
    return out.astype(np.float32).reshape(B, H, W, C)
